# revision 1
# baseline (speedup 1.0000x reference)
"""Trainium2 Bass kernel for a small dense transformer (Bigram model).

Model: B=4, T=2048, E=256, H=4 heads (HS=64), L=3 layers, V=32000 vocab.
logits = lm_head(trunk(tok_emb[idx] + pos_emb))  -> [4, 2048, 32000] f32.

Sharding over 8 NeuronCores: core c handles batch b = c//2 and vocab half
vh = c%2.  Each core runs the full trunk for its batch (replicated across
the pair -- trunk is small next to the lm_head) and then computes
logits[b, :, vh*16000:(vh+1)*16000] = x @ Wlm[:, half].  The [B,T,V] output
write dominates; it is split 8 ways so each core writes 131 MB.

Layout strategy inside a core:
  - Residual stream x kept natural [t(128-part) x E] in SBUF for LayerNorm.
  - LN output transposed via PE into hT [E(part) x T] so QKV/MLP matmuls
    run as lhsT=W (or lhsT=hT) with fp32r at full PE rate.
  - Attention scores computed transposed S_T[s, t] = k . q so softmax sum
    comes from a ones-column matmul and P_T feeds the y^T matmul directly;
    no [T,T] transpose needed.  exp() has no max-subtraction (scores are
    O(1) by construction), masked blocks are skipped / zeroed via
    affine_select after exp.
  - ln gains/biases and the attention 1/sqrt(E) scale are folded into the
    weight matrices on the host.  All additive biases in this problem are
    zero; nonzero biases are handled on the host (lm bias) or rejected.
"""

import numpy as np

P = 128
T = 2048
E = 256
H = 4
HS = 64
L = 3
V = 32000
VSH = V // 2  # vocab half per core
NT = T // P  # 16 token tiles of 128
TT = 512  # attention t-block
NTT = T // TT  # 4
NLM = 500  # lm_head vocab tile
NLMT = VSH // NLM  # 32
EPS = 1e-5

_CACHE = {}


def _build_program(cfg=None):
    import concourse.bass as bass
    import concourse.mybir as mybir
    import concourse.tile as tile
    from concourse import bacc
    from concourse.masks import make_identity
    from contextlib import ExitStack

    cfg = cfg or {}
    att_bufs = cfg.get("att_bufs", 2)
    mm_bufs = cfg.get("mm_bufs", 6)
    y_bufs = cfg.get("y_bufs", 2)
    pt_bufs = cfg.get("pt_bufs", 6)
    pair_att = cfg.get("pair_att", False)
    split_pools = cfg.get("split_pools", True)
    lm_bufs = cfg.get("lm_bufs", 3)
    lmw_bufs = cfg.get("lmw_bufs", 4)
    ob_bufs = cfg.get("ob_bufs", 6)
    relu_dve = cfg.get("relu_dve", False)
    n_layers = cfg.get("n_layers", L)
    skip_lm = cfg.get("skip_lm", False)
    w_bufs = cfg.get("w_bufs", 1)
    qk_copy_split = cfg.get("qk_copy_split", True)
    obg = cfg.get("obg", 1)
    ti_outer = cfg.get("ti_outer", False)
    a_bufs = cfg.get("a_bufs", 2)
    y_drain = cfg.get("y_drain", False)
    lm_shared_psum = cfg.get("lm_shared_psum", False)
    work_bufs = cfg.get("work_bufs", 4)
    tr_copy_split = cfg.get("tr_copy_split", True)
    xn_gpsimd = cfg.get("xn_gpsimd", True)
    v_copy_act = cfg.get("v_copy_act", False)
    emb_gpsimd = cfg.get("emb_gpsimd", False)
    mask_dve = cfg.get("mask_dve", False)
    pair_res = cfg.get("pair_res", True)

    F32 = mybir.dt.float32
    F32R = mybir.dt.float32r
    I32 = mybir.dt.int32
    AF = mybir.ActivationFunctionType
    ALU = mybir.AluOpType

    nc = bacc.Bacc()
    idx32 = nc.declare_dram_parameter("idx32", [T], I32, isOutput=False)
    tok = nc.declare_dram_parameter("tok_emb", [V, E], F32, isOutput=False)
    pos = nc.declare_dram_parameter("pos_emb", [T, E], F32, isOutput=False)
    wq = nc.declare_dram_parameter("wq", [L, E, E], F32R, isOutput=False)
    wk = nc.declare_dram_parameter("wk", [L, E, E], F32R, isOutput=False)
    wv = nc.declare_dram_parameter("wv", [L, E, E], F32R, isOutput=False)
    wp = nc.declare_dram_parameter("wp", [L, E, E], F32R, isOutput=False)
    w1 = nc.declare_dram_parameter("w1", [L, E, 4 * E], F32R, isOutput=False)
    w2 = nc.declare_dram_parameter("w2", [L, 4 * E, E], F32R, isOutput=False)
    wlm = nc.declare_dram_parameter("wlm", [E, VSH], F32R, isOutput=False)
    out = nc.declare_dram_parameter("out", [T, VSH], F32, isOutput=True)

    with tile.TileContext(nc) as tc, ExitStack() as ctx:
        const = ctx.enter_context(tc.tile_pool(name="const", bufs=1))
        res = ctx.enter_context(tc.tile_pool(name="res", bufs=1))
        # trunk-phase pools, closed before the lm head when split_pools
        tk = ctx.enter_context(ExitStack())
        wpool = tk.enter_context(tc.tile_pool(name="wts", bufs=w_bufs))
        work = tk.enter_context(tc.tile_pool(name="work", bufs=work_bufs))
        apool = tk.enter_context(tc.tile_pool(name="apool", bufs=a_bufs))
        ppool = tk.enter_context(tc.tile_pool(name="ppool", bufs=pt_bufs))
        # PSUM budget: 8 banks total.
        ps_att = (
            tk.enter_context(tc.tile_pool(name="ps_att", bufs=att_bufs, space="PSUM"))
            if pair_att
            else None
        )
        _psctx = ctx if lm_shared_psum else tk
        ps_mm = _psctx.enter_context(
            tc.tile_pool(name="ps_mm", bufs=mm_bufs, space="PSUM")
        )
        ps_y = _psctx.enter_context(
            tc.tile_pool(name="ps_y", bufs=y_bufs, space="PSUM")
        )
        ps_tr = ps_mm

        ident = const.tile([P, P], F32, tag="ident", name="ident")
        make_identity(nc, ident)
        causal_m = None
        if mask_dve:
            # upper-triangular (incl diag) ones: keep iff ft >= p
            causal_m = const.tile([P, P], F32, tag="cmask", name="cmask")
            nc.vector.memset(causal_m, 1.0)
            nc.gpsimd.affine_select(
                out=causal_m,
                in_=causal_m,
                compare_op=ALU.is_ge,
                fill=0.0,
                base=0,
                channel_multiplier=-1,
                pattern=[[1, P]],
            )
        eps_t = const.tile([P, 1], F32, tag="eps", name="eps")
        nc.vector.memset(eps_t, EPS)

        idx_sb = const.tile([P, NT], I32, tag="idx", name="idx_sb")
        nc.sync.dma_start(out=idx_sb, in_=idx32.rearrange("(n p) -> p n", p=P))

        # ---- embedding: x = tok_emb[idx] + pos_emb ----
        x_sb = res.tile([P, NT, E], F32, tag="x", name="x_sb")
        for n in range(NT):
            xg = work.tile([P, E], F32, tag="xg", name="xg")
            nc.gpsimd.indirect_dma_start(
                out=xg,
                out_offset=None,
                in_=tok[:],
                in_offset=bass.IndirectOffsetOnAxis(ap=idx_sb[:, n : n + 1], axis=0),
            )
            pt = work.tile([P, E], F32, tag="pos", name="pos_t")
            nc.sync.dma_start(out=pt, in_=pos[n * P : (n + 1) * P, :])
            (nc.gpsimd if emb_gpsimd else nc.vector).tensor_add(
                out=x_sb[:, n, :], in0=xg, in1=pt
            )

        def ln_transpose(dstT, do_ln=True):
            """LayerNorm x_sb (no affine: folded into weights) then write the
            transpose into dstT [P, 2, T] ([E-part, token] layout)."""
            for n in range(NT):
                if do_ln:
                    stats = work.tile([P, 6], F32, tag="stats", name="stats")
                    nc.vector.bn_stats(out=stats, in_=x_sb[:, n, :])
                    mv = work.tile([P, 2], F32, tag="mv", name="mv")
                    nc.vector.bn_aggr(out=mv, in_=stats)
                    std = work.tile([P, 1], F32, tag="std", name="std")
                    nc.scalar.activation(
                        out=std, in_=mv[:, 1:2], func=AF.Sqrt, bias=eps_t, scale=1.0
                    )
                    rstd = work.tile([P, 1], F32, tag="rstd", name="rstd")
                    nc.vector.reciprocal(out=rstd, in_=std)
                    xn = work.tile([P, E], F32, tag="xn", name="xn")
                    _ts_eng = nc.gpsimd if xn_gpsimd else nc.vector
                    _ts_eng.tensor_scalar(
                        out=xn,
                        in0=x_sb[:, n, :],
                        scalar1=mv[:, 0:1],
                        scalar2=rstd,
                        op0=ALU.subtract,
                        op1=ALU.mult,
                    )
                else:
                    xn = x_sb[:, n, :]
                for c in range(2):
                    ptr = ps_tr.tile([P, 512], F32, tag="mm", name="ps_trp")
                    nc.tensor.transpose(
                        ptr[:, :P], xn[:, c * P : (c + 1) * P], ident
                    )
                    if tr_copy_split and c == 1:
                        nc.scalar.copy(
                            out=dstT[:, c, n * P : (n + 1) * P], in_=ptr[:, :P]
                        )
                    else:
                        nc.vector.tensor_copy(
                            out=dstT[:, c, n * P : (n + 1) * P], in_=ptr[:, :P]
                        )

        # ---- transformer layers ----
        for l in range(n_layers):
            wq_sb = wpool.tile([P, 2, E], F32R, tag="wq", name="wq_sb")
            nc.sync.dma_start(out=wq_sb, in_=wq[l].rearrange("(c p) n -> p c n", p=P))
            wk_sb = wpool.tile([P, 2, E], F32R, tag="wk", name="wk_sb")
            nc.sync.dma_start(out=wk_sb, in_=wk[l].rearrange("(c p) n -> p c n", p=P))
            wv_sb = wpool.tile([P, 2, E], F32R, tag="wv", name="wv_sb")
            nc.sync.dma_start(out=wv_sb, in_=wv[l].rearrange("(c p) n -> p c n", p=P))
            wp_sb = wpool.tile([P, 2, E], F32R, tag="wp", name="wp_sb")
            nc.sync.dma_start(out=wp_sb, in_=wp[l].rearrange("(c p) n -> p c n", p=P))
            w1_sb = wpool.tile([P, 2, 4 * E], F32R, tag="w1", name="w1_sb")
            nc.sync.dma_start(out=w1_sb, in_=w1[l].rearrange("(c p) n -> p c n", p=P))
            w2_sb = wpool.tile([P, 8, E], F32R, tag="w2", name="w2_sb")
            nc.sync.dma_start(out=w2_sb, in_=w2[l].rearrange("(c p) n -> p c n", p=P))

            hT = res.tile([P, 2, T], F32R, tag="hT", name="hT")
            ln_transpose(hT)

            # q^T, k^T: [E(part, 2 chunks), T]  (heads h: chunk h//2, rows (h%2)*64)
            qT = res.tile([P, 2, T], F32R, tag="qT", name="qT")
            kT = res.tile([P, 2, T], F32R, tag="kT", name="kT")
            for w_sb, dstT in ((wq_sb, qT), (wk_sb, kT)):
                for co in range(2):
                    for ti in range(NTT):
                        ps = ps_mm.tile([P, 512], F32, tag="mm", name="ps_qk")
                        for ci in range(2):
                            nc.tensor.matmul(
                                ps,
                                lhsT=w_sb[:, ci, co * P : (co + 1) * P],
                                rhs=hT[:, ci, ti * TT : (ti + 1) * TT],
                                start=(ci == 0),
                                stop=(ci == 1),
                            )
                        if qk_copy_split and ti % 2 == 0:
                            nc.vector.tensor_copy(
                                out=dstT[:, co, ti * TT : (ti + 1) * TT], in_=ps
                            )
                        else:
                            nc.scalar.copy(
                                out=dstT[:, co, ti * TT : (ti + 1) * TT], in_=ps
                            )

            # v natural [t(part), head, 65] with ones column for the softmax sum
            v_sb = res.tile([P, NT, H, HS + 1], F32R, tag="v", name="v_sb")
            nc.vector.memset(v_sb[:, :, :, HS : HS + 1].bitcast(F32), 1.0)
            for n in range(NT):
                ps = ps_mm.tile([P, 512], F32, tag="mm", name="ps_v")
                for ci in range(2):
                    nc.tensor.matmul(
                        ps[:, :E],
                        lhsT=hT[:, ci, n * P : (n + 1) * P],
                        rhs=wv_sb[:, ci, :],
                        start=(ci == 0),
                        stop=(ci == 1),
                    )
                if v_copy_act:
                    nc.scalar.copy(
                        out=v_sb[:, n, :, 0:HS],
                        in_=ps[:, :E].rearrange("p (h d) -> p h d", h=H),
                    )
                else:
                    nc.vector.tensor_copy(
                        out=v_sb[:, n, :, 0:HS],
                        in_=ps[:, :E].rearrange("p (h d) -> p h d", h=H),
                    )

            # attention, transposed-score flash style
            yT = res.tile([P, 2, T], F32R, tag="yT", name="yT")
            W = 2 if pair_att else 1
            for ti_h in range(NTT * H):
                if ti_outer:
                    ti, h = divmod(ti_h, H)
                else:
                    h, ti = divmod(ti_h, NTT)
                if True:
                    kc, ko = h // 2, (h % 2) * HS
                    t0 = ti * TT
                    ngrp = (4 * ti + 4) // W  # s-chunk groups (W x 128 each)
                    py = ps_y.tile([HS + 1, TT], F32, tag="y", name="ps_yacc")
                    for pi in range(ngrp):
                        s0 = pi * W * P
                        if pair_att:
                            ps = ps_att.tile([P, W, TT], F32, tag="att", name="ps_s")
                        else:
                            ps = ps_mm.tile([P, W, TT], F32, tag="mm", name="ps_s")
                        # j-index of each chunk relative to the diagonal:
                        # chunk si covers s in [128si, 128si+128); only
                        # t >= s is live, i.e. columns [128j:TT) where
                        # j = si - 4*ti (for diagonal chunks, j in 0..3).
                        for c in range(W):
                            si = W * pi + c
                            j = si - 4 * ti
                            d0 = P * j if j > 0 else 0  # first live column
                            nc.tensor.matmul(
                                ps[:, c, d0:],
                                lhsT=kT[ko : ko + HS, kc, s0 + c * P : s0 + (c + 1) * P],
                                rhs=qT[ko : ko + HS, kc, t0 + d0 : t0 + TT],
                                start=True,
                                stop=True,
                            )
                        pT = ppool.tile([P, W, TT], F32R, tag="pT", name="pT")
                        for c in range(W):
                            si = W * pi + c
                            j = si - 4 * ti
                            d0 = P * j if j > 0 else 0
                            nc.scalar.activation(
                                out=pT[:, c, d0:], in_=ps[:, c, d0:], func=AF.Exp
                            )
                            if j >= 0:
                                # mask only the 128-col diagonal sub-block
                                if mask_dve:
                                    nc.vector.tensor_mul(
                                        out=pT[:, c, d0 : d0 + P],
                                        in0=pT[:, c, d0 : d0 + P],
                                        in1=causal_m,
                                    )
                                else:
                                    nc.gpsimd.affine_select(
                                        out=pT[:, c, d0 : d0 + P],
                                        in_=pT[:, c, d0 : d0 + P],
                                        compare_op=ALU.is_ge,
                                        fill=0.0,
                                        base=0,
                                        channel_multiplier=-1,
                                        pattern=[[1, P]],
                                    )
                            nc.tensor.matmul(
                                py[:, d0:],
                                lhsT=v_sb[:, si, h, :],
                                rhs=pT[:, c, d0:],
                                start=(si == 0),
                                stop=(si == 4 * ti + 3),
                            )
                    if y_drain:
                        ysb = work.tile([HS + 1, TT], F32, tag="ysb", name="ysb")
                        nc.scalar.copy(out=ysb, in_=py)
                        ysrc = ysb
                    else:
                        ysrc = py
                    linv = work.tile([1, TT], F32, tag="linv", name="linv")
                    nc.vector.reciprocal(out=linv, in_=ysrc[HS : HS + 1, :])
                    linb = work.tile([HS, TT], F32, tag="linb", name="linb")
                    nc.gpsimd.partition_broadcast(linb, linv)
                    nc.vector.tensor_mul(
                        out=yT[ko : ko + HS, kc, t0 : t0 + TT],
                        in0=ysrc[0:HS, :],
                        in1=linb,
                    )

            # proj + residual
            if pair_res:
                for n2 in range(NT // 2):
                    ps = ps_mm.tile([P, 512], F32, tag="mm", name="ps_proj")
                    for k in range(2):
                        n = 2 * n2 + k
                        for ci in range(2):
                            nc.tensor.matmul(
                                ps[:, k * E : (k + 1) * E],
                                lhsT=yT[:, ci, n * P : (n + 1) * P],
                                rhs=wp_sb[:, ci, :],
                                start=(ci == 0),
                                stop=(ci == 1),
                            )
                    nc.vector.tensor_add(
                        out=x_sb[:, 2 * n2 : 2 * n2 + 2, :],
                        in0=x_sb[:, 2 * n2 : 2 * n2 + 2, :],
                        in1=ps.rearrange("p (k e) -> p k e", k=2),
                    )
            else:
                for n in range(NT):
                    ps = ps_mm.tile([P, 512], F32, tag="mm", name="ps_proj")
                    for ci in range(2):
                        nc.tensor.matmul(
                            ps[:, :E],
                            lhsT=yT[:, ci, n * P : (n + 1) * P],
                            rhs=wp_sb[:, ci, :],
                            start=(ci == 0),
                            stop=(ci == 1),
                        )
                    nc.vector.tensor_add(
                        out=x_sb[:, n, :], in0=x_sb[:, n, :], in1=ps[:, :E]
                    )

            # MLP
            h2T = res.tile([P, 2, T], F32R, tag="hT", name="h2T")
            ln_transpose(h2T)
            for ti in range(NTT):
                aT = apool.tile([P, 8, TT], F32R, tag="aT", name="aT")
                for m in range(8):
                    ps = ps_mm.tile([P, TT], F32, tag="mm", name="ps_a")
                    for ci in range(2):
                        nc.tensor.matmul(
                            ps,
                            lhsT=w1_sb[:, ci, m * P : (m + 1) * P],
                            rhs=h2T[:, ci, ti * TT : (ti + 1) * TT],
                            start=(ci == 0),
                            stop=(ci == 1),
                        )
                    if relu_dve:
                        nc.vector.tensor_scalar_max(out=aT[:, m, :], in0=ps, scalar1=0.0)
                    else:
                        nc.scalar.activation(out=aT[:, m, :], in_=ps, func=AF.Relu)
                if pair_res:
                    for k2 in range(2):
                        ps2 = ps_mm.tile([P, 512], F32, tag="mm", name="ps_o2")
                        for k in range(2):
                            n = ti * 4 + k2 * 2 + k
                            for m in range(8):
                                nc.tensor.matmul(
                                    ps2[:, k * E : (k + 1) * E],
                                    lhsT=aT[:, m, (k2 * 2 + k) * P : (k2 * 2 + k + 1) * P],
                                    rhs=w2_sb[:, m, :],
                                    start=(m == 0),
                                    stop=(m == 7),
                                )
                        n0 = ti * 4 + k2 * 2
                        nc.vector.tensor_add(
                            out=x_sb[:, n0 : n0 + 2, :],
                            in0=x_sb[:, n0 : n0 + 2, :],
                            in1=ps2.rearrange("p (k e) -> p k e", k=2),
                        )
                else:
                    for k in range(4):
                        n = ti * 4 + k
                        ps2 = ps_mm.tile([P, 512], F32, tag="mm", name="ps_o2")
                        for m in range(8):
                            nc.tensor.matmul(
                                ps2[:, :E],
                                lhsT=aT[:, m, k * P : (k + 1) * P],
                                rhs=w2_sb[:, m, :],
                                start=(m == 0),
                                stop=(m == 7),
                            )
                        nc.vector.tensor_add(
                            out=x_sb[:, n, :], in0=x_sb[:, n, :], in1=ps2[:, :E]
                        )

        # ---- lm head ----
        xfT = res.tile([P, 2, T], F32R, tag="hT", name="xfT")
        ln_transpose(xfT, do_ln=False)
        if split_pools:
            tk.close()
        lmw = ctx.enter_context(tc.tile_pool(name="lmw", bufs=lmw_bufs))
        opool = ctx.enter_context(tc.tile_pool(name="opool", bufs=ob_bufs))
        ps_lm = (
            None
            if lm_shared_psum
            else ctx.enter_context(
                tc.tile_pool(name="ps_lm", bufs=lm_bufs, space="PSUM")
            )
        )
        for nv2 in range(0 if skip_lm else NLMT // 2):
            wl = lmw.tile([P, 2, 2 * NLM], F32R, tag="wlm", name="wl")
            nc.sync.dma_start(
                out=wl,
                in_=wlm[:, nv2 * 2 * NLM : (nv2 + 1) * 2 * NLM].rearrange(
                    "(c p) n -> p c n", p=P
                ),
            )
            for g in range(NT // obg):
                ob = opool.tile([P, obg, 2 * NLM], F32, tag="ob", name="ob")
                for k in range(obg):
                    n = g * obg + k
                    if lm_shared_psum:
                        dst = ob[:, k, :].rearrange("p (j n) -> p j n", j=2)
                        for j in range(2):
                            pa = ps_mm.tile([P, 512], F32, tag="mm", name="ps_lmu")
                            for ci in range(2):
                                nc.tensor.matmul(
                                    pa[:, :NLM],
                                    lhsT=xfT[:, ci, n * P : (n + 1) * P],
                                    rhs=wl[:, ci, j * NLM : (j + 1) * NLM],
                                    start=(ci == 0),
                                    stop=(ci == 1),
                                )
                            if (n + nv2 + j) % 2 == 0:
                                nc.vector.tensor_copy(
                                    out=dst[:, j, :], in_=pa[:, :NLM]
                                )
                            else:
                                nc.scalar.copy(out=dst[:, j, :], in_=pa[:, :NLM])
                    else:
                        pa = ps_lm.tile([P, 2, TT], F32, tag="lm", name="ps_lm")
                        for j in range(2):
                            for ci in range(2):
                                nc.tensor.matmul(
                                    pa[:, j, :NLM],
                                    lhsT=xfT[:, ci, n * P : (n + 1) * P],
                                    rhs=wl[:, ci, j * NLM : (j + 1) * NLM],
                                    start=(ci == 0),
                                    stop=(ci == 1),
                                )
                        dst = ob[:, k, :].rearrange("p (j n) -> p j n", j=2)
                        if (n + nv2) % 2 == 0:
                            nc.vector.tensor_copy(out=dst, in_=pa[:, :, :NLM])
                        else:
                            nc.scalar.copy(out=dst, in_=pa[:, :, :NLM])
                nc.sync.dma_start(
                    out=out[g * obg * P : (g + 1) * obg * P,
                            nv2 * 2 * NLM : (nv2 + 1) * 2 * NLM]
                    .rearrange("(k p) n -> p k n", p=P),
                    in_=ob,
                )

    nc.compile()
    return nc


TRACE = False
LAST_RESULT = None


def kernel(**inputs):
    from concourse.bass_utils import run_bass_kernel_spmd

    global LAST_RESULT

    idx = np.ascontiguousarray(np.asarray(inputs["idx"]).astype(np.int32))  # [4, T]
    tok_emb = np.ascontiguousarray(np.asarray(inputs["tok_emb"], np.float32))
    pos_emb = np.ascontiguousarray(np.asarray(inputs["pos_emb"], np.float32))
    Wq = np.asarray(inputs["Wq"], np.float32)
    Wk = np.asarray(inputs["Wk"], np.float32)
    Wv = np.asarray(inputs["Wv"], np.float32)
    Wproj = np.asarray(inputs["Wproj"], np.float32)
    bproj = np.asarray(inputs["bproj"], np.float32)
    ln1_g = np.asarray(inputs["ln1_g"], np.float32)
    ln1_b = np.asarray(inputs["ln1_b"], np.float32)
    W1 = np.asarray(inputs["W1"], np.float32)
    b1 = np.asarray(inputs["b1"], np.float32)
    W2 = np.asarray(inputs["W2"], np.float32)
    b2 = np.asarray(inputs["b2"], np.float32)
    ln2_g = np.asarray(inputs["ln2_g"], np.float32)
    ln2_b = np.asarray(inputs["ln2_b"], np.float32)
    Wlm = np.asarray(inputs["Wlm"], np.float32)
    blm = np.asarray(inputs["blm"], np.float32)

    # This kernel folds the LN affine into the weights; additive biases after
    # the matmuls are zero in this model (asserted).  The lm bias is applied
    # on the host if nonzero.
    for name, b in (("bproj", bproj), ("b1", b1), ("b2", b2)):
        assert np.all(b == 0.0), f"{name} must be zero for this kernel"
    for name, b in (("ln1_b", ln1_b), ("ln2_b", ln2_b)):
        assert np.all(b == 0.0), f"{name} must be zero for this kernel"

    scale = 1.0 / np.sqrt(np.float32(E))
    wq_f = np.ascontiguousarray(ln1_g[:, :, None] * Wq * scale)  # [L,E,E]
    wk_f = np.ascontiguousarray(ln1_g[:, :, None] * Wk)
    wv_f = np.ascontiguousarray(ln1_g[:, :, None] * Wv)
    wp_f = np.ascontiguousarray(Wproj)
    w1_f = np.ascontiguousarray(ln2_g[:, :, None] * W1)
    w2_f = np.ascontiguousarray(W2)

    if "nc" not in _CACHE:
        _CACHE["nc"] = _build_program()
    nc = _CACHE["nc"]

    common = {
        "tok_emb": tok_emb,
        "pos_emb": pos_emb,
        "wq": wq_f,
        "wk": wk_f,
        "wv": wv_f,
        "wp": wp_f,
        "w1": w1_f,
        "w2": w2_f,
    }
    in_maps = []
    for c in range(8):
        b, vh = c // 2, c % 2
        m = dict(common)
        m["idx32"] = np.ascontiguousarray(idx[b])
        m["wlm"] = np.ascontiguousarray(Wlm[:, vh * VSH : (vh + 1) * VSH])
        in_maps.append(m)

    r = run_bass_kernel_spmd(nc, in_maps, list(range(8)), trace=TRACE)
    LAST_RESULT = r

    B = idx.shape[0]
    logits = np.empty((B, T, V), np.float32)
    for c in range(8):
        b, vh = c // 2, c % 2
        logits[b, :, vh * VSH : (vh + 1) * VSH] = r.results[c]["out"]
    if np.any(blm != 0.0):
        logits += blm
    return logits



# revision 30
# speedup vs baseline: 1.2941x; 1.2941x over previous
"""Trainium2 Bass kernel for a small dense transformer (Bigram model).

Model: B=4, T=2048, E=256, H=4 heads (HS=64), L=3 layers, V=32000 vocab.
logits = lm_head(trunk(tok_emb[idx] + pos_emb))  -> [4, 2048, 32000] f32.

Sharding over 8 NeuronCores: core c handles batch b = c//2 and vocab half
vh = c%2.  Each core runs the full trunk for its batch (replicated across
the pair -- trunk is small next to the lm_head) and then computes
logits[b, :, vh*16000:(vh+1)*16000] = x @ Wlm[:, half].

v2 changes vs v1 (baseline 912852 ns):
  - Output is written to DRAM as float16 (range |logits| < 1, fp16 rounding
    ~5e-4 relative) and upcast to f32 on the host: halves the dominant
    131MB/core output-write DMA traffic.
  - lm_head weights + final activations in bf16: halves the 16MB wlm load.
  - Attention internals (q^T, k^T, v, softmax weights) in bf16: bf16
    matmuls run 1 cycle/row at any width (fp32r needs >=256), removing the
    4x penalty on the 128-wide diagonal blocks.
  - exp() over PAIRS of 128-row s-chunks (one Act op per [P,2,TT] psum
    tile): halves Act instruction count in attention, the engine that was
    100% busy there.
  - Unified [P,2,512] psum tiles (3 bufs) + [65,512] y-accumulators (2
    bufs) = 8 banks; all copies/drains are 1000+ element ops.
  - Engine rebalance: relu alternates Act/DVE, psum->sbuf copies alternate
    DVE/Pool, lm drains round-robin Act/DVE/Pool.
  - ln gains and the attention 1/sqrt(E) scale are folded into the weights
    on the host; all additive biases are zero (asserted) or host-applied.
"""

import numpy as np

P = 128
T = 2048
E = 256
H = 4
HS = 64
L = 3
V = 32000
VSH = V // 2  # vocab half per core
NT = T // P  # 16 token tiles of 128
TT = 512  # attention t-block
NTT = T // TT  # 4
NLM = 500  # lm_head vocab tile
NLMT = VSH // NLM  # 32
EPS = 1e-5

_CACHE = {}


def _build_program(cfg=None):
    import concourse.bass as bass
    import concourse.mybir as mybir
    import concourse.tile as tile
    from concourse import bacc
    from concourse.masks import make_identity
    from contextlib import ExitStack

    cfg = cfg or {}
    ps2_bufs = cfg.get("ps2_bufs", 3)
    y_bufs = cfg.get("y_bufs", 2)
    pt_bufs = cfg.get("pt_bufs", 4)
    a_bufs = cfg.get("a_bufs", 2)
    work_bufs = cfg.get("work_bufs", 4)
    lm_bufs = cfg.get("lm_bufs", 4)
    lmw_bufs = cfg.get("lmw_bufs", 4)
    ob_bufs = cfg.get("ob_bufs", 6)
    n_layers = cfg.get("n_layers", L)
    skip_lm = cfg.get("skip_lm", False)
    ti_outer = cfg.get("ti_outer", False)

    F32 = mybir.dt.float32
    F32R = mybir.dt.float32r
    BF16 = mybir.dt.bfloat16
    F16 = mybir.dt.float16
    I32 = mybir.dt.int32
    AF = mybir.ActivationFunctionType
    ALU = mybir.AluOpType

    nc = bacc.Bacc()
    idx32 = nc.declare_dram_parameter("idx32", [T], I32, isOutput=False)
    tok = nc.declare_dram_parameter("tok_emb", [V, E], F32, isOutput=False)
    pos = nc.declare_dram_parameter("pos_emb", [T, E], F32, isOutput=False)
    wq = nc.declare_dram_parameter("wq", [L, E, E], F32R, isOutput=False)
    wk = nc.declare_dram_parameter("wk", [L, E, E], F32R, isOutput=False)
    wv = nc.declare_dram_parameter("wv", [L, E, E], F32R, isOutput=False)
    wp = nc.declare_dram_parameter("wp", [L, E, E], F32R, isOutput=False)
    w1 = nc.declare_dram_parameter("w1", [L, E, 4 * E], F32R, isOutput=False)
    w2 = nc.declare_dram_parameter("w2", [L, 4 * E, E], BF16, isOutput=False)
    wlm = nc.declare_dram_parameter("wlm", [E, VSH], BF16, isOutput=False)
    out = nc.declare_dram_parameter("out", [T, VSH], F16, isOutput=True)

    with tile.TileContext(nc) as tc, ExitStack() as ctx:
        const = ctx.enter_context(tc.tile_pool(name="const", bufs=1))
        res = ctx.enter_context(tc.tile_pool(name="res", bufs=1))
        # trunk-phase pools, closed before the lm head
        tk = ctx.enter_context(ExitStack())
        wpool = tk.enter_context(tc.tile_pool(name="wts", bufs=1))
        work = tk.enter_context(tc.tile_pool(name="work", bufs=work_bufs))
        apool = tk.enter_context(tc.tile_pool(name="apool", bufs=a_bufs))
        ppool = tk.enter_context(tc.tile_pool(name="ppool", bufs=pt_bufs))
        # PSUM budget: 8 banks. ps2 tiles are [P,2,TT] f32 = 2 banks each.
        ps2 = tk.enter_context(tc.tile_pool(name="ps2", bufs=ps2_bufs, space="PSUM"))
        ps_y = tk.enter_context(tc.tile_pool(name="ps_y", bufs=y_bufs, space="PSUM"))

        ident = const.tile([P, P], F32, tag="ident", name="ident")
        make_identity(nc, ident)
        eps_t = const.tile([P, 1], F32, tag="eps", name="eps")
        nc.vector.memset(eps_t, EPS)

        idx_sb = const.tile([P, NT], I32, tag="idx", name="idx_sb")
        nc.sync.dma_start(out=idx_sb, in_=idx32.rearrange("(n p) -> p n", p=P))

        # round-robin engine pickers (per-category counters)
        _rr = {"cp": 0, "relu": 0, "drain": 0}

        def rr(kind, engines):
            e = engines[_rr[kind] % len(engines)]
            _rr[kind] += 1
            return e

        # ---- embedding: x = tok_emb[idx] + pos_emb ----
        x_sb = res.tile([P, NT, E], F32, tag="x", name="x_sb")
        pt = res.tile([P, NT, E], F32, tag="pos", name="pos_t")
        nc.sync.dma_start(out=pt, in_=pos.rearrange("(n p) e -> p n e", p=P))
        for g in range(NT // 4):
            sl = slice(4 * g, 4 * g + 4)
            nc.gpsimd.indirect_dma_start(
                out=x_sb[:, sl, :],
                out_offset=None,
                in_=tok[:],
                in_offset=bass.IndirectOffsetOnAxis(ap=idx_sb[:, sl], axis=0),
            )
            nc.vector.tensor_add(
                out=x_sb[:, sl, :], in0=x_sb[:, sl, :], in1=pt[:, sl, :]
            )

        def ln_transpose(dstT, do_ln=True):
            """LayerNorm x_sb (affine folded into weights) then write the
            transpose into dstT [P, 2, T] ([E-part, token] layout)."""
            for n in range(NT):
                if do_ln:
                    stats = work.tile([P, 6], F32, tag="stats", name="stats")
                    nc.vector.bn_stats(out=stats, in_=x_sb[:, n, :])
                    mv = work.tile([P, 2], F32, tag="mv", name="mv")
                    nc.vector.bn_aggr(out=mv, in_=stats)
                    std = work.tile([P, 1], F32, tag="std", name="std")
                    nc.scalar.activation(
                        out=std, in_=mv[:, 1:2], func=AF.Sqrt, bias=eps_t, scale=1.0
                    )
                    rstd = work.tile([P, 1], F32, tag="rstd", name="rstd")
                    nc.vector.reciprocal(out=rstd, in_=std)
                    xn = work.tile([P, E], F32, tag="xn", name="xn")
                    nc.gpsimd.tensor_scalar(
                        out=xn,
                        in0=x_sb[:, n, :],
                        scalar1=mv[:, 0:1],
                        scalar2=rstd,
                        op0=ALU.subtract,
                        op1=ALU.mult,
                    )
                else:
                    xn = x_sb[:, n, :]
                ptr = ps2.tile([P, 2, TT], F32, tag="s2", name="ps_trp")
                for c in range(2):
                    nc.tensor.transpose(
                        ptr[:, c, :P], xn[:, c * P : (c + 1) * P], ident
                    )
                rr("cp", (nc.vector, nc.gpsimd)).tensor_copy(
                    out=dstT[:, :, n * P : (n + 1) * P], in_=ptr[:, :, :P]
                )

        # ---- transformer layers ----
        for l in range(n_layers):
            wq_sb = wpool.tile([P, 2, E], F32R, tag="wq", name="wq_sb")
            nc.sync.dma_start(out=wq_sb, in_=wq[l].rearrange("(c p) n -> p c n", p=P))
            wk_sb = wpool.tile([P, 2, E], F32R, tag="wk", name="wk_sb")
            nc.sync.dma_start(out=wk_sb, in_=wk[l].rearrange("(c p) n -> p c n", p=P))
            wv_sb = wpool.tile([P, 2, E], F32R, tag="wv", name="wv_sb")
            nc.sync.dma_start(out=wv_sb, in_=wv[l].rearrange("(c p) n -> p c n", p=P))
            wp_sb = wpool.tile([P, 2, E], F32R, tag="wp", name="wp_sb")
            nc.sync.dma_start(out=wp_sb, in_=wp[l].rearrange("(c p) n -> p c n", p=P))
            w1_sb = wpool.tile([P, 2, 4 * E], F32R, tag="w1", name="w1_sb")
            nc.sync.dma_start(out=w1_sb, in_=w1[l].rearrange("(c p) n -> p c n", p=P))
            w2_sb = wpool.tile([P, 8, E], BF16, tag="w2", name="w2_sb")
            nc.sync.dma_start(out=w2_sb, in_=w2[l].rearrange("(c p) n -> p c n", p=P))

            hT = res.tile([P, 2, T], F32R, tag="hT", name="hT")
            ln_transpose(hT)

            # q^T, k^T in bf16: [E(part, 2 chunks), T]
            # (heads h: chunk h//2, rows (h%2)*64)
            qT = res.tile([P, 2, T], BF16, tag="qT", name="qT")
            kT = res.tile([P, 2, T], BF16, tag="kT", name="kT")
            for w_sb, dstT in ((wq_sb, qT), (wk_sb, kT)):
                for ti in range(NTT):
                    ps = ps2.tile([P, 2, TT], F32, tag="s2", name="ps_qk")
                    for co in range(2):
                        for ci in range(2):
                            nc.tensor.matmul(
                                ps[:, co, :],
                                lhsT=w_sb[:, ci, co * P : (co + 1) * P],
                                rhs=hT[:, ci, ti * TT : (ti + 1) * TT],
                                start=(ci == 0),
                                stop=(ci == 1),
                            )
                    rr("cp", (nc.vector, nc.gpsimd)).tensor_copy(
                        out=dstT[:, :, ti * TT : (ti + 1) * TT], in_=ps
                    )

            # v natural [t(part), head, 65] bf16 with ones column for the
            # softmax sum
            v_sb = res.tile([P, NT, H, HS + 1], BF16, tag="v", name="v_sb")
            nc.vector.memset(v_sb[:, :, :, HS : HS + 1].bitcast(BF16), 1.0)
            for n2 in range(NT // 2):
                ps = ps2.tile([P, 2, TT], F32, tag="s2", name="ps_v")
                for c in range(2):
                    n = 2 * n2 + c
                    for ci in range(2):
                        nc.tensor.matmul(
                            ps[:, c, :E],
                            lhsT=hT[:, ci, n * P : (n + 1) * P],
                            rhs=wv_sb[:, ci, :],
                            start=(ci == 0),
                            stop=(ci == 1),
                        )
                nc.vector.tensor_copy(
                    out=v_sb[:, 2 * n2 : 2 * n2 + 2, :, 0:HS],
                    in_=ps[:, :, :E].rearrange("p c (h d) -> p c h d", h=H),
                )

            # attention, transposed-score flash style, bf16 weights
            yT = res.tile([P, 2, T], F32R, tag="yT", name="yT")
            for ti_h in range(NTT * H):
                if ti_outer:
                    ti, h = divmod(ti_h, H)
                else:
                    h, ti = divmod(ti_h, NTT)
                kc, ko = h // 2, (h % 2) * HS
                t0 = ti * TT
                py = ps_y.tile([HS + 1, TT], F32, tag="y", name="ps_yacc")
                npair = 2 * ti + 2
                for pi in range(npair):
                    ps = ps2.tile([P, 2, TT], F32, tag="s2", name="ps_s")
                    pT = ppool.tile([P, 2, TT], BF16, tag="pT", name="pT")
                    d0s = []
                    for c in range(2):
                        si = 2 * pi + c
                        j = si - 4 * ti
                        d0 = P * j if j > 0 else 0  # first live column
                        d0s.append(d0)
                        nc.tensor.matmul(
                            ps[:, c, d0:],
                            lhsT=kT[ko : ko + HS, kc, si * P : (si + 1) * P],
                            rhs=qT[ko : ko + HS, kc, t0 + d0 : t0 + TT],
                            start=True,
                            stop=True,
                        )
                    # one exp per pair; for diagonal-group pairs the region
                    # left of each chunk's d0 is causally dead (excluded from
                    # the y matmul or masked below), so exp'ing the stale psum
                    # there is harmless
                    nc.scalar.activation(
                        out=pT[:, :, d0s[0] :], in_=ps[:, :, d0s[0] :], func=AF.Exp
                    )
                    if pi >= 2 * ti:
                        for c in range(2):
                            d0 = d0s[c]
                            nc.gpsimd.affine_select(
                                out=pT[:, c, d0 : d0 + P],
                                in_=pT[:, c, d0 : d0 + P],
                                compare_op=ALU.is_ge,
                                fill=0.0,
                                base=0,
                                channel_multiplier=-1,
                                pattern=[[1, P]],
                            )
                    for c in range(2):
                        si = 2 * pi + c
                        d0 = d0s[c]
                        nc.tensor.matmul(
                            py[:, d0:],
                            lhsT=v_sb[:, si, h, :],
                            rhs=pT[:, c, d0:],
                            start=(si == 0),
                            stop=(si == 4 * ti + 3),
                        )
                linv = work.tile([1, TT], F32, tag="linv", name="linv")
                nc.vector.reciprocal(out=linv, in_=py[HS : HS + 1, :])
                linb = work.tile([HS, TT], F32, tag="linb", name="linb")
                nc.gpsimd.partition_broadcast(linb, linv)
                nc.vector.tensor_mul(
                    out=yT[ko : ko + HS, kc, t0 : t0 + TT],
                    in0=py[0:HS, :],
                    in1=linb,
                )

            # proj + residual
            for n2 in range(NT // 2):
                ps = ps2.tile([P, 2, TT], F32, tag="s2", name="ps_proj")
                for k in range(2):
                    n = 2 * n2 + k
                    for ci in range(2):
                        nc.tensor.matmul(
                            ps[:, k, :E],
                            lhsT=yT[:, ci, n * P : (n + 1) * P],
                            rhs=wp_sb[:, ci, :],
                            start=(ci == 0),
                            stop=(ci == 1),
                        )
                nc.vector.tensor_add(
                    out=x_sb[:, 2 * n2 : 2 * n2 + 2, :],
                    in0=x_sb[:, 2 * n2 : 2 * n2 + 2, :],
                    in1=ps[:, :, :E],
                )

            # MLP
            h2T = res.tile([P, 2, T], F32R, tag="hT", name="h2T")
            ln_transpose(h2T)
            for ti in range(NTT):
                aT = apool.tile([P, 8, TT], BF16, tag="aT", name="aT")
                for q in range(4):
                    ps = ps2.tile([P, 2, TT], F32, tag="s2", name="ps_a")
                    for c in range(2):
                        m = 2 * q + c
                        for ci in range(2):
                            nc.tensor.matmul(
                                ps[:, c, :],
                                lhsT=w1_sb[:, ci, m * P : (m + 1) * P],
                                rhs=h2T[:, ci, ti * TT : (ti + 1) * TT],
                                start=(ci == 0),
                                stop=(ci == 1),
                            )
                    eng = rr("relu", (nc.scalar, nc.vector))
                    if eng is nc.scalar:
                        eng.activation(
                            out=aT[:, 2 * q : 2 * q + 2, :], in_=ps, func=AF.Relu
                        )
                    else:
                        eng.tensor_scalar_max(
                            out=aT[:, 2 * q : 2 * q + 2, :], in0=ps, scalar1=0.0
                        )
                for k2 in range(2):
                    ps2t = ps2.tile([P, 2, TT], F32, tag="s2", name="ps_o2")
                    for k in range(2):
                        n = ti * 4 + k2 * 2 + k
                        for m in range(8):
                            nc.tensor.matmul(
                                ps2t[:, k, :E],
                                lhsT=aT[:, m, (k2 * 2 + k) * P : (k2 * 2 + k + 1) * P],
                                rhs=w2_sb[:, m, :],
                                start=(m == 0),
                                stop=(m == 7),
                            )
                    n0 = ti * 4 + k2 * 2
                    nc.vector.tensor_add(
                        out=x_sb[:, n0 : n0 + 2, :],
                        in0=x_sb[:, n0 : n0 + 2, :],
                        in1=ps2t[:, :, :E],
                    )

        # ---- lm head ----
        xfT = res.tile([P, 2, T], BF16, tag="xfT", name="xfT")
        ln_transpose(xfT, do_ln=False)
        tk.close()
        lmw = ctx.enter_context(tc.tile_pool(name="lmw", bufs=lmw_bufs))
        opool = ctx.enter_context(tc.tile_pool(name="opool", bufs=ob_bufs))
        ps_lm = ctx.enter_context(tc.tile_pool(name="ps_lm", bufs=lm_bufs, space="PSUM"))
        for nv2 in range(0 if skip_lm else NLMT // 2):
            wl = lmw.tile([P, 2, 2 * NLM], BF16, tag="wlm", name="wl")
            nc.sync.dma_start(
                out=wl,
                in_=wlm[:, nv2 * 2 * NLM : (nv2 + 1) * 2 * NLM].rearrange(
                    "(c p) n -> p c n", p=P
                ),
            )
            for n in range(NT):
                pa = ps_lm.tile([P, 2, NLM], F32, tag="lm", name="ps_lm")
                for j in range(2):
                    for ci in range(2):
                        nc.tensor.matmul(
                            pa[:, j, :],
                            lhsT=xfT[:, ci, n * P : (n + 1) * P],
                            rhs=wl[:, ci, j * NLM : (j + 1) * NLM],
                            start=(ci == 0),
                            stop=(ci == 1),
                        )
                ob = opool.tile([P, 2, NLM], F16, tag="ob", name="ob")
                eng = rr("drain", (nc.scalar, nc.vector, nc.gpsimd))
                if eng is nc.scalar:
                    eng.copy(out=ob, in_=pa)
                else:
                    eng.tensor_copy(out=ob, in_=pa)
                nc.sync.dma_start(
                    out=out[n * P : (n + 1) * P,
                            nv2 * 2 * NLM : (nv2 + 1) * 2 * NLM]
                    .rearrange("p (j n) -> p j n", j=2),
                    in_=ob,
                )

    nc.compile()
    return nc


TRACE = False
LAST_RESULT = None


def kernel(**inputs):
    from concourse.bass_utils import run_bass_kernel_spmd
    import ml_dtypes

    global LAST_RESULT

    idx = np.ascontiguousarray(np.asarray(inputs["idx"]).astype(np.int32))  # [4, T]
    tok_emb = np.ascontiguousarray(np.asarray(inputs["tok_emb"], np.float32))
    pos_emb = np.ascontiguousarray(np.asarray(inputs["pos_emb"], np.float32))
    Wq = np.asarray(inputs["Wq"], np.float32)
    Wk = np.asarray(inputs["Wk"], np.float32)
    Wv = np.asarray(inputs["Wv"], np.float32)
    Wproj = np.asarray(inputs["Wproj"], np.float32)
    bproj = np.asarray(inputs["bproj"], np.float32)
    ln1_g = np.asarray(inputs["ln1_g"], np.float32)
    ln1_b = np.asarray(inputs["ln1_b"], np.float32)
    W1 = np.asarray(inputs["W1"], np.float32)
    b1 = np.asarray(inputs["b1"], np.float32)
    W2 = np.asarray(inputs["W2"], np.float32)
    b2 = np.asarray(inputs["b2"], np.float32)
    ln2_g = np.asarray(inputs["ln2_g"], np.float32)
    ln2_b = np.asarray(inputs["ln2_b"], np.float32)
    Wlm = np.asarray(inputs["Wlm"], np.float32)
    blm = np.asarray(inputs["blm"], np.float32)

    # This kernel folds the LN affine into the weights; additive biases after
    # the matmuls are zero in this model (asserted).  The lm bias is applied
    # on the host if nonzero.
    for name, b in (("bproj", bproj), ("b1", b1), ("b2", b2)):
        assert np.all(b == 0.0), f"{name} must be zero for this kernel"
    for name, b in (("ln1_b", ln1_b), ("ln2_b", ln2_b)):
        assert np.all(b == 0.0), f"{name} must be zero for this kernel"

    scale = 1.0 / np.sqrt(np.float32(E))
    wq_f = np.ascontiguousarray(ln1_g[:, :, None] * Wq * scale)  # [L,E,E]
    wk_f = np.ascontiguousarray(ln1_g[:, :, None] * Wk)
    wv_f = np.ascontiguousarray(ln1_g[:, :, None] * Wv)
    wp_f = np.ascontiguousarray(Wproj)
    w1_f = np.ascontiguousarray(ln2_g[:, :, None] * W1)
    w2_f = np.ascontiguousarray(W2.astype(ml_dtypes.bfloat16))
    wlm_bf = np.ascontiguousarray(Wlm.astype(ml_dtypes.bfloat16))

    if "nc" not in _CACHE:
        _CACHE["nc"] = _build_program()
    nc = _CACHE["nc"]

    common = {
        "tok_emb": tok_emb,
        "pos_emb": pos_emb,
        "wq": wq_f,
        "wk": wk_f,
        "wv": wv_f,
        "wp": wp_f,
        "w1": w1_f,
        "w2": w2_f,
    }
    in_maps = []
    for c in range(8):
        b, vh = c // 2, c % 2
        m = dict(common)
        m["idx32"] = np.ascontiguousarray(idx[b])
        m["wlm"] = np.ascontiguousarray(wlm_bf[:, vh * VSH : (vh + 1) * VSH])
        in_maps.append(m)

    r = run_bass_kernel_spmd(nc, in_maps, list(range(8)), trace=TRACE)
    LAST_RESULT = r

    B = idx.shape[0]
    logits = np.empty((B, T, V), np.float32)
    for c in range(8):
        b, vh = c // 2, c % 2
        logits[b, :, vh * VSH : (vh + 1) * VSH] = np.asarray(
            r.results[c]["out"], np.float32
        )
    if np.any(blm != 0.0):
        logits += blm
    return logits


# revision 34
# speedup vs baseline: 1.3375x; 1.0335x over previous
"""Trainium2 Bass kernel for a small dense transformer (Bigram model).

Model: B=4, T=2048, E=256, H=4 heads (HS=64), L=3 layers, V=32000 vocab.
logits = lm_head(trunk(tok_emb[idx] + pos_emb))  -> [4, 2048, 32000] f32.

Sharding over 8 NeuronCores: core c handles batch b = c//2 and vocab half
vh = c%2.  Each core runs the full trunk for its batch (replicated across
the pair -- trunk is small next to the lm_head) and then computes
logits[b, :, vh*16000:(vh+1)*16000] = x @ Wlm[:, half].

v2 changes vs v1 (baseline 912852 ns):
  - Output is written to DRAM as float16 (range |logits| < 1, fp16 rounding
    ~5e-4 relative) and upcast to f32 on the host: halves the dominant
    131MB/core output-write DMA traffic.
  - lm_head weights + final activations in bf16: halves the 16MB wlm load.
  - Attention internals (q^T, k^T, v, softmax weights) in bf16: bf16
    matmuls run 1 cycle/row at any width (fp32r needs >=256), removing the
    4x penalty on the 128-wide diagonal blocks.
  - exp() over PAIRS of 128-row s-chunks (one Act op per [P,2,TT] psum
    tile): halves Act instruction count in attention, the engine that was
    100% busy there.
  - Unified [P,2,512] psum tiles (3 bufs) + [65,512] y-accumulators (2
    bufs) = 8 banks; all copies/drains are 1000+ element ops.
  - Engine rebalance: relu alternates Act/DVE, psum->sbuf copies alternate
    DVE/Pool, lm drains round-robin Act/DVE/Pool.
  - ln gains and the attention 1/sqrt(E) scale are folded into the weights
    on the host; all additive biases are zero (asserted) or host-applied.
"""

import numpy as np

P = 128
T = 2048
E = 256
H = 4
HS = 64
L = 3
V = 32000
VSH = V // 2  # vocab half per core
NT = T // P  # 16 token tiles of 128
TT = 512  # attention t-block
NTT = T // TT  # 4
NLM = 500  # lm_head vocab tile
NLMT = VSH // NLM  # 32
EPS = 1e-5

_CACHE = {}


def _build_program(cfg=None):
    import concourse.bass as bass
    import concourse.mybir as mybir
    import concourse.tile as tile
    from concourse import bacc
    from concourse.masks import make_identity
    from contextlib import ExitStack

    cfg = cfg or {}
    ps2_bufs = cfg.get("ps2_bufs", 3)
    y_bufs = cfg.get("y_bufs", 2)
    pt_bufs = cfg.get("pt_bufs", 4)
    a_bufs = cfg.get("a_bufs", 2)
    work_bufs = cfg.get("work_bufs", 4)
    lm_bufs = cfg.get("lm_bufs", 4)
    lmw_bufs = cfg.get("lmw_bufs", 4)
    ob_bufs = cfg.get("ob_bufs", 6)
    n_layers = cfg.get("n_layers", L)
    skip_lm = cfg.get("skip_lm", False)
    debug_x = cfg.get("debug_x", False)
    ti_outer = cfg.get("ti_outer", False)

    F32 = mybir.dt.float32
    F32R = mybir.dt.float32r
    BF16 = mybir.dt.bfloat16
    F16 = mybir.dt.float16
    I32 = mybir.dt.int32
    AF = mybir.ActivationFunctionType
    ALU = mybir.AluOpType

    nc = bacc.Bacc()
    idx32 = nc.declare_dram_parameter("idx32", [T], I32, isOutput=False)
    tok = nc.declare_dram_parameter("tok_emb", [V, E], F32, isOutput=False)
    pos = nc.declare_dram_parameter("pos_emb", [T, E], F32, isOutput=False)
    wq = nc.declare_dram_parameter("wq", [L, E, E], F32R, isOutput=False)
    wk = nc.declare_dram_parameter("wk", [L, E, E], F32R, isOutput=False)
    wv = nc.declare_dram_parameter("wv", [L, E, E], F32R, isOutput=False)
    wp = nc.declare_dram_parameter("wp", [L, E, E], F32R, isOutput=False)
    w1 = nc.declare_dram_parameter("w1", [L, E, 4 * E], F32R, isOutput=False)
    w2 = nc.declare_dram_parameter("w2", [L, 4 * E, E], BF16, isOutput=False)
    wlm = nc.declare_dram_parameter("wlm", [E, VSH], BF16, isOutput=False)
    out = nc.declare_dram_parameter("out", [T, VSH], F16, isOutput=True)
    if cfg.get("debug_x"):
        dbg = nc.declare_dram_parameter("dbg_x", [T, E], mybir.dt.float32, isOutput=True)
    else:
        dbg = None

    with tile.TileContext(nc) as tc, ExitStack() as ctx:
        const = ctx.enter_context(tc.tile_pool(name="const", bufs=1))
        res = ctx.enter_context(tc.tile_pool(name="res", bufs=1))
        # trunk-phase pools, closed before the lm head
        tk = ctx.enter_context(ExitStack())
        wpool = tk.enter_context(tc.tile_pool(name="wts", bufs=1))
        work = tk.enter_context(tc.tile_pool(name="work", bufs=work_bufs))
        apool = tk.enter_context(tc.tile_pool(name="apool", bufs=a_bufs))
        ppool = tk.enter_context(tc.tile_pool(name="ppool", bufs=pt_bufs))
        # PSUM budget: 8 banks. ps2 tiles are [P,2,TT] f32 = 2 banks each.
        ps2 = tk.enter_context(tc.tile_pool(name="ps2", bufs=ps2_bufs, space="PSUM"))
        ps_y = tk.enter_context(tc.tile_pool(name="ps_y", bufs=y_bufs, space="PSUM"))

        ident = const.tile([P, P], F32, tag="ident", name="ident")
        make_identity(nc, ident)
        eps_t = const.tile([P, 1], F32, tag="eps", name="eps")
        nc.vector.memset(eps_t, EPS)

        idx_sb = const.tile([P, NT], I32, tag="idx", name="idx_sb")
        nc.sync.dma_start(out=idx_sb, in_=idx32.rearrange("(n p) -> p n", p=P))

        # round-robin engine pickers (per-category counters)
        _rr = {"cp": 0, "relu": 0, "drain": 0}

        def rr(kind, engines):
            e = engines[_rr[kind] % len(engines)]
            _rr[kind] += 1
            return e

        # ---- embedding: x = tok_emb[idx] + pos_emb ----
        x_sb = res.tile([P, NT, E], F32, tag="x", name="x_sb")
        pt = res.tile([P, NT, E], F32, tag="pos", name="pos_t")
        nc.sync.dma_start(out=pt, in_=pos.rearrange("(n p) e -> p n e", p=P))
        for g in range(NT // 4):
            sl = slice(4 * g, 4 * g + 4)
            nc.gpsimd.indirect_dma_start(
                out=x_sb[:, sl, :],
                out_offset=None,
                in_=tok[:],
                in_offset=bass.IndirectOffsetOnAxis(ap=idx_sb[:, sl], axis=0),
            )
            nc.vector.tensor_add(
                out=x_sb[:, sl, :], in0=x_sb[:, sl, :], in1=pt[:, sl, :]
            )

        def ln_transpose(dstT, do_ln=True):
            """LayerNorm x_sb (affine folded into weights) then write the
            transpose into dstT [P, 2, T] ([E-part, token] layout)."""
            for n in range(NT):
                if do_ln:
                    stats = work.tile([P, 6], F32, tag="stats", name="stats")
                    nc.vector.bn_stats(out=stats, in_=x_sb[:, n, :])
                    mv = work.tile([P, 2], F32, tag="mv", name="mv")
                    nc.vector.bn_aggr(out=mv, in_=stats)
                    std = work.tile([P, 1], F32, tag="std", name="std")
                    nc.scalar.activation(
                        out=std, in_=mv[:, 1:2], func=AF.Sqrt, bias=eps_t, scale=1.0
                    )
                    rstd = work.tile([P, 1], F32, tag="rstd", name="rstd")
                    nc.vector.reciprocal(out=rstd, in_=std)
                    xn = work.tile([P, E], F32, tag="xn", name="xn")
                    nc.gpsimd.tensor_scalar(
                        out=xn,
                        in0=x_sb[:, n, :],
                        scalar1=mv[:, 0:1],
                        scalar2=rstd,
                        op0=ALU.subtract,
                        op1=ALU.mult,
                    )
                else:
                    xn = x_sb[:, n, :]
                ptr = ps2.tile([P, 2, TT], F32, tag="s2", name="ps_trp")
                for c in range(2):
                    nc.tensor.transpose(
                        ptr[:, c, :P], xn[:, c * P : (c + 1) * P], ident
                    )
                eng = rr("cp", (nc.vector, nc.scalar))
                if eng is nc.scalar:
                    eng.copy(out=dstT[:, :, n * P : (n + 1) * P], in_=ptr[:, :, :P])
                else:
                    eng.tensor_copy(
                        out=dstT[:, :, n * P : (n + 1) * P], in_=ptr[:, :, :P]
                    )

        # ---- transformer layers ----
        for l in range(n_layers):
            wq_sb = wpool.tile([P, 2, E], F32R, tag="wq", name="wq_sb")
            nc.sync.dma_start(out=wq_sb, in_=wq[l].rearrange("(c p) n -> p c n", p=P))
            wk_sb = wpool.tile([P, 2, E], F32R, tag="wk", name="wk_sb")
            nc.sync.dma_start(out=wk_sb, in_=wk[l].rearrange("(c p) n -> p c n", p=P))
            wv_sb = wpool.tile([P, 2, E], F32R, tag="wv", name="wv_sb")
            nc.sync.dma_start(out=wv_sb, in_=wv[l].rearrange("(c p) n -> p c n", p=P))
            wp_sb = wpool.tile([P, 2, E], F32R, tag="wp", name="wp_sb")
            nc.sync.dma_start(out=wp_sb, in_=wp[l].rearrange("(c p) n -> p c n", p=P))
            w1_sb = wpool.tile([P, 2, 4 * E], F32R, tag="w1", name="w1_sb")
            nc.sync.dma_start(out=w1_sb, in_=w1[l].rearrange("(c p) n -> p c n", p=P))
            w2_sb = wpool.tile([P, 8, E], BF16, tag="w2", name="w2_sb")
            nc.sync.dma_start(out=w2_sb, in_=w2[l].rearrange("(c p) n -> p c n", p=P))

            hT = res.tile([P, 2, T], F32R, tag="hT", name="hT")
            ln_transpose(hT)

            # q^T, k^T in bf16: [E(part, 2 chunks), T]
            # (heads h: chunk h//2, rows (h%2)*64)
            qT = res.tile([P, 2, T], BF16, tag="qT", name="qT")
            kT = res.tile([P, 2, T], BF16, tag="kT", name="kT")
            for w_sb, dstT in ((wq_sb, qT), (wk_sb, kT)):
                for ti in range(NTT):
                    ps = ps2.tile([P, 2, TT], F32, tag="s2", name="ps_qk")
                    for co in range(2):
                        for ci in range(2):
                            nc.tensor.matmul(
                                ps[:, co, :],
                                lhsT=w_sb[:, ci, co * P : (co + 1) * P],
                                rhs=hT[:, ci, ti * TT : (ti + 1) * TT],
                                start=(ci == 0),
                                stop=(ci == 1),
                            )
                    eng = rr("cp", (nc.vector, nc.scalar))
                    if eng is nc.scalar:
                        eng.copy(out=dstT[:, :, ti * TT : (ti + 1) * TT], in_=ps)
                    else:
                        eng.tensor_copy(
                            out=dstT[:, :, ti * TT : (ti + 1) * TT], in_=ps
                        )

            # v natural [t(part), head, 65] bf16 with ones column for the
            # softmax sum
            v_sb = res.tile([P, NT, H, HS + 1], BF16, tag="v", name="v_sb")
            nc.vector.memset(v_sb[:, :, :, HS : HS + 1].bitcast(BF16), 1.0)
            for n2 in range(NT // 2):
                ps = ps2.tile([P, 2, TT], F32, tag="s2", name="ps_v")
                for c in range(2):
                    n = 2 * n2 + c
                    for ci in range(2):
                        nc.tensor.matmul(
                            ps[:, c, :E],
                            lhsT=hT[:, ci, n * P : (n + 1) * P],
                            rhs=wv_sb[:, ci, :],
                            start=(ci == 0),
                            stop=(ci == 1),
                        )
                nc.vector.tensor_copy(
                    out=v_sb[:, 2 * n2 : 2 * n2 + 2, :, 0:HS],
                    in_=ps[:, :, :E].rearrange("p c (h d) -> p c h d", h=H),
                )

            # attention, transposed-score flash style, bf16 weights
            yT = res.tile([P, 2, T], F32R, tag="yT", name="yT")
            for ti_h in range(NTT * H):
                if ti_outer:
                    ti, h = divmod(ti_h, H)
                else:
                    h, ti = divmod(ti_h, NTT)
                kc, ko = h // 2, (h % 2) * HS
                t0 = ti * TT
                py = ps_y.tile([HS + 1, TT], F32, tag="y", name="ps_yacc")
                npair = 2 * ti + 2
                for pi in range(npair):
                    ps = ps2.tile([P, 2, TT], F32, tag="s2", name="ps_s")
                    pT = ppool.tile([P, 2, TT], BF16, tag="pT", name="pT")
                    d0s = []
                    for c in range(2):
                        si = 2 * pi + c
                        j = si - 4 * ti
                        d0 = P * j if j > 0 else 0  # first live column
                        d0s.append(d0)
                        nc.tensor.matmul(
                            ps[:, c, d0:],
                            lhsT=kT[ko : ko + HS, kc, si * P : (si + 1) * P],
                            rhs=qT[ko : ko + HS, kc, t0 + d0 : t0 + TT],
                            start=True,
                            stop=True,
                        )
                    # one exp per pair; for diagonal-group pairs the region
                    # left of each chunk's d0 is causally dead (excluded from
                    # the y matmul or masked below), so exp'ing the stale psum
                    # there is harmless
                    nc.scalar.activation(
                        out=pT[:, :, d0s[0] :], in_=ps[:, :, d0s[0] :], func=AF.Exp
                    )
                    if pi >= 2 * ti:
                        for c in range(2):
                            d0 = d0s[c]
                            nc.gpsimd.affine_select(
                                out=pT[:, c, d0 : d0 + P],
                                in_=pT[:, c, d0 : d0 + P],
                                compare_op=ALU.is_ge,
                                fill=0.0,
                                base=0,
                                channel_multiplier=-1,
                                pattern=[[1, P]],
                            )
                    for c in range(2):
                        si = 2 * pi + c
                        d0 = d0s[c]
                        nc.tensor.matmul(
                            py[:, d0:],
                            lhsT=v_sb[:, si, h, :],
                            rhs=pT[:, c, d0:],
                            start=(si == 0),
                            stop=(si == 4 * ti + 3),
                        )
                linv = work.tile([1, TT], F32, tag="linv", name="linv")
                nc.vector.reciprocal(out=linv, in_=py[HS : HS + 1, :])
                linb = work.tile([HS, TT], F32, tag="linb", name="linb")
                nc.gpsimd.partition_broadcast(linb, linv)
                nc.vector.tensor_mul(
                    out=yT[ko : ko + HS, kc, t0 : t0 + TT],
                    in0=py[0:HS, :],
                    in1=linb,
                )

            # proj + residual
            for n2 in range(NT // 2):
                ps = ps2.tile([P, 2, TT], F32, tag="s2", name="ps_proj")
                for k in range(2):
                    n = 2 * n2 + k
                    for ci in range(2):
                        nc.tensor.matmul(
                            ps[:, k, :E],
                            lhsT=yT[:, ci, n * P : (n + 1) * P],
                            rhs=wp_sb[:, ci, :],
                            start=(ci == 0),
                            stop=(ci == 1),
                        )
                nc.vector.tensor_add(
                    out=x_sb[:, 2 * n2 : 2 * n2 + 2, :],
                    in0=x_sb[:, 2 * n2 : 2 * n2 + 2, :],
                    in1=ps[:, :, :E],
                )

            # MLP
            h2T = res.tile([P, 2, T], F32R, tag="hT", name="h2T")
            ln_transpose(h2T)
            for ti in range(NTT):
                aT = apool.tile([P, 8, TT], BF16, tag="aT", name="aT")
                for q in range(4):
                    ps = ps2.tile([P, 2, TT], F32, tag="s2", name="ps_a")
                    for c in range(2):
                        m = 2 * q + c
                        for ci in range(2):
                            nc.tensor.matmul(
                                ps[:, c, :],
                                lhsT=w1_sb[:, ci, m * P : (m + 1) * P],
                                rhs=h2T[:, ci, ti * TT : (ti + 1) * TT],
                                start=(ci == 0),
                                stop=(ci == 1),
                            )
                    eng = rr("relu", (nc.scalar, nc.vector))
                    if eng is nc.scalar:
                        eng.activation(
                            out=aT[:, 2 * q : 2 * q + 2, :], in_=ps, func=AF.Relu
                        )
                    else:
                        eng.tensor_scalar_max(
                            out=aT[:, 2 * q : 2 * q + 2, :], in0=ps, scalar1=0.0
                        )
                for k2 in range(2):
                    ps2t = ps2.tile([P, 2, TT], F32, tag="s2", name="ps_o2")
                    for k in range(2):
                        n = ti * 4 + k2 * 2 + k
                        for m in range(8):
                            nc.tensor.matmul(
                                ps2t[:, k, :E],
                                lhsT=aT[:, m, (k2 * 2 + k) * P : (k2 * 2 + k + 1) * P],
                                rhs=w2_sb[:, m, :],
                                start=(m == 0),
                                stop=(m == 7),
                            )
                    n0 = ti * 4 + k2 * 2
                    nc.vector.tensor_add(
                        out=x_sb[:, n0 : n0 + 2, :],
                        in0=x_sb[:, n0 : n0 + 2, :],
                        in1=ps2t[:, :, :E],
                    )

        # ---- lm head ----
        xfT = res.tile([P, 2, T], BF16, tag="xfT", name="xfT")
        ln_transpose(xfT, do_ln=False)
        if debug_x:
            nc.sync.dma_start(
                out=dbg.rearrange("(n p) e -> p n e", p=P), in_=x_sb
            )
        tk.close()
        lmw = ctx.enter_context(tc.tile_pool(name="lmw", bufs=lmw_bufs))
        opool = ctx.enter_context(tc.tile_pool(name="opool", bufs=ob_bufs))
        ps_lm = ctx.enter_context(tc.tile_pool(name="ps_lm", bufs=lm_bufs, space="PSUM"))
        for nv2 in range(0 if skip_lm else NLMT // 2):
            wl = lmw.tile([P, 2, 2 * NLM], BF16, tag="wlm", name="wl")
            nc.sync.dma_start(
                out=wl,
                in_=wlm[:, nv2 * 2 * NLM : (nv2 + 1) * 2 * NLM].rearrange(
                    "(c p) n -> p c n", p=P
                ),
            )
            for n in range(NT):
                # full 2KB banks: a [P,2,NLM] tile would put the j=1 matmul
                # output across a psum bank boundary (silent corruption)
                pa = ps_lm.tile([P, 2, TT], F32, tag="lm", name="ps_lm")
                # ci outer so consecutive matmuls share lhsT (halves the
                # Ldweights traffic on the saturated PE sequencer)
                for ci in range(2):
                    for j in range(2):
                        nc.tensor.matmul(
                            pa[:, j, :NLM],
                            lhsT=xfT[:, ci, n * P : (n + 1) * P],
                            rhs=wl[:, ci, j * NLM : (j + 1) * NLM],
                            start=(ci == 0),
                            stop=(ci == 1),
                        )
                ob = opool.tile([P, 2, NLM], F16, tag="ob", name="ob")
                eng = rr("drain", (nc.scalar, nc.vector))
                if eng is nc.scalar:
                    eng.copy(out=ob, in_=pa[:, :, :NLM])
                else:
                    eng.tensor_copy(out=ob, in_=pa[:, :, :NLM])
                nc.sync.dma_start(
                    out=out[n * P : (n + 1) * P,
                            nv2 * 2 * NLM : (nv2 + 1) * 2 * NLM]
                    .rearrange("p (j n) -> p j n", j=2),
                    in_=ob,
                )

    nc.compile()
    return nc


TRACE = False
LAST_RESULT = None


def kernel(**inputs):
    from concourse.bass_utils import run_bass_kernel_spmd
    import ml_dtypes

    global LAST_RESULT

    idx = np.ascontiguousarray(np.asarray(inputs["idx"]).astype(np.int32))  # [4, T]
    tok_emb = np.ascontiguousarray(np.asarray(inputs["tok_emb"], np.float32))
    pos_emb = np.ascontiguousarray(np.asarray(inputs["pos_emb"], np.float32))
    Wq = np.asarray(inputs["Wq"], np.float32)
    Wk = np.asarray(inputs["Wk"], np.float32)
    Wv = np.asarray(inputs["Wv"], np.float32)
    Wproj = np.asarray(inputs["Wproj"], np.float32)
    bproj = np.asarray(inputs["bproj"], np.float32)
    ln1_g = np.asarray(inputs["ln1_g"], np.float32)
    ln1_b = np.asarray(inputs["ln1_b"], np.float32)
    W1 = np.asarray(inputs["W1"], np.float32)
    b1 = np.asarray(inputs["b1"], np.float32)
    W2 = np.asarray(inputs["W2"], np.float32)
    b2 = np.asarray(inputs["b2"], np.float32)
    ln2_g = np.asarray(inputs["ln2_g"], np.float32)
    ln2_b = np.asarray(inputs["ln2_b"], np.float32)
    Wlm = np.asarray(inputs["Wlm"], np.float32)
    blm = np.asarray(inputs["blm"], np.float32)

    # This kernel folds the LN affine into the weights; additive biases after
    # the matmuls are zero in this model (asserted).  The lm bias is applied
    # on the host if nonzero.
    for name, b in (("bproj", bproj), ("b1", b1), ("b2", b2)):
        assert np.all(b == 0.0), f"{name} must be zero for this kernel"
    for name, b in (("ln1_b", ln1_b), ("ln2_b", ln2_b)):
        assert np.all(b == 0.0), f"{name} must be zero for this kernel"

    scale = 1.0 / np.sqrt(np.float32(E))
    wq_f = np.ascontiguousarray(ln1_g[:, :, None] * Wq * scale)  # [L,E,E]
    wk_f = np.ascontiguousarray(ln1_g[:, :, None] * Wk)
    wv_f = np.ascontiguousarray(ln1_g[:, :, None] * Wv)
    wp_f = np.ascontiguousarray(Wproj)
    w1_f = np.ascontiguousarray(ln2_g[:, :, None] * W1)
    w2_f = np.ascontiguousarray(W2.astype(ml_dtypes.bfloat16))
    wlm_bf = np.ascontiguousarray(Wlm.astype(ml_dtypes.bfloat16))

    if "nc" not in _CACHE:
        _CACHE["nc"] = _build_program()
    nc = _CACHE["nc"]

    common = {
        "tok_emb": tok_emb,
        "pos_emb": pos_emb,
        "wq": wq_f,
        "wk": wk_f,
        "wv": wv_f,
        "wp": wp_f,
        "w1": w1_f,
        "w2": w2_f,
    }
    in_maps = []
    for c in range(8):
        b, vh = c // 2, c % 2
        m = dict(common)
        m["idx32"] = np.ascontiguousarray(idx[b])
        m["wlm"] = np.ascontiguousarray(wlm_bf[:, vh * VSH : (vh + 1) * VSH])
        in_maps.append(m)

    r = run_bass_kernel_spmd(nc, in_maps, list(range(8)), trace=TRACE)
    LAST_RESULT = r

    B = idx.shape[0]
    logits = np.empty((B, T, V), np.float32)
    for c in range(8):
        b, vh = c // 2, c % 2
        logits[b, :, vh * VSH : (vh + 1) * VSH] = np.asarray(
            r.results[c]["out"], np.float32
        )
    if np.any(blm != 0.0):
        logits += blm
    return logits
